# revision 49
# baseline (speedup 1.0000x reference)
"""Trainium2 Bass kernel: single-channel 15x15 cross-correlation (pad=1,
stride=1) of a 4096x4096 fp32 image, + scalar bias.

Strategy: 2D-patch packing, 6 matmul passes per 128-pixel block
---------------------------------------------------------------
The image is space-to-depth'd on the host: partition dim packs a 16x8
(row x col) patch, so SBUF column 516*t + g holds xpad[16t + r, 8g + co]
at partition r*8 + co.  An output block of 16x8 = 128 pixels out[(io,
jo)] at column group n accumulates 6 matmuls (a in {0,1} row-patches x
b in {0,1,2} col-shifts): stationary A_ab[(r, co), (io, jo)] =
W[16a + r - io, 8b + co - jo], moving operand = input band t+a shifted
by b groups.  Each (di, dj) tap appears in exactly one (a, b, r, co), so
6 passes replace the 15 banded passes of the naive Toeplitz scheme (the
6-pass count is optimal: a block needs (16+14)*(8+14) = 660 distinct
inputs and a stream column carries at most 128).

Work per core: 32 bands (512 output rows) x 512 column groups -> 192
matmuls of [K=128, M=128] x [128, 512], all operands bf16 (PSUM f32,
rel err ~2.6e-3).  bf16 matters twice: LDWEIGHTS drops to ~100 ns and
hides fully under the ~173 ns PE pipeline window (fp32r weight loads
are 2 B/col and stick out, 234->280 ns per matmul), and wire traffic
halves.  Steady state measured 215 ns per matmul = 512 cols at 2.4 GHz.

Scheduling details, all trace-driven:
  - input rides the SP HWDGE ring as 7 size-ramped chunk DMAs (the
    33 per-band DMAs' ring-slot semaphores cross-paced with the output
    ring and stalled the PE; big chunks also move at ~300 GB/s vs
    ~90 GB/s for 1 KB-element transfers);
  - weights ride the Activation ring as two 98 KB transfers sized so
    wts[0..2] land just before the first real matmul needs them;
  - all 8 PSUM banks cycle as single-band accumulators (bufs=8), so
    the PE never waits on the DVE bias-add evacuation;
  - output stores go 4 bands per DMA (4 KB elements) split across both
    HWDGE rings, and the last band stores as two 64 KB halves to keep
    the post-compute tail short;
  - throwaway matmuls on memset data bridge the fixed ~7 us engine
    preamble until input lands, so the PE p-state (0.65/1.2/2.4 GHz
    ramp) is at full clock when the first real matmul issues.
Host pre-pads/reshapes inputs and un-shuffles outputs; halos ride in
each core's input slice, no collectives.  Measured: 146.8 us (banded
baseline) -> ~58 us, rel err 2.6e-3.
"""

import os

import numpy as np

KH = KW = 15
PAD = 1
H = W = 4096
OUT = H + 2 * PAD - KH + 1  # 4084
NCORES = 8
BR = 16  # output rows per band (and patch rows)
BC = 8  # col-group width (and patch cols)
NBAND = 32  # bands per core -> 512 output rows per core
NG = 512  # output col groups per stream (512*8 = 4096 >= 4084 cols)
GROUPS = NG + 4  # col groups per band (2 extra for b-shift, pad to 516)
NPASS = 6  # 2 row-patches x 3 col-shifts
ROWS_PC = NBAND * BR  # 512 output rows per core
XR_BANDS = NBAND + 1  # input bands per core (one extra for the halo)
XPAD_R = NCORES * ROWS_PC + BR  # 4112 padded input rows
XPAD_C = GROUPS * BC  # 4128 padded input cols
CH_NB = [2, 2, 3, 4, 8, 8, 6]  # input-chunk sizes in bands (sum = 33)
N_WARMUP = 6
OG = 4  # bands per output-store group (4 KB DMA elements for ring rate)
NOG = NBAND // OG

LAST_RESULT = None  # BassKernelResults of the most recent run (for test.py)


def _patch_drain():
    """walrus's CTRL_NO instruction struct holds very few semaphore waits;
    Tile's kernel-tail drain aggregates one wait per logical processor and
    overflows it.  Spread the waits across 1-wait-per-nop SP instructions."""
    import concourse.mybir as mybir
    import concourse.tile as tile
    from concourse.vector_clock import ScopedClock

    def _split_drain_and_barrier(self, tick_clock, wait_clock):
        nc = self.nc
        probe = nc.sync.nop(nofuse=True)
        wait_clock.add_sem_waits(
            probe.ins, ScopedClock({None: tick_clock.global_clock})
        )
        si = probe.ins.sync_info
        if si is not None and len(si.on_wait) > 1:
            waits = list(si.on_wait)
            probe.ins.sync_info = mybir.SyncInfo(
                on_wait=waits[:1], on_update=list(si.on_update)
            )
            for w in waits[1:]:
                extra = nc.sync.nop(nofuse=True)
                extra.ins.sync_info = mybir.SyncInfo(on_wait=[w], on_update=[])
        nc.sync.drain()
        # The stock exit path does barrier -> semaphore cleanup -> barrier
        # (~8us).  This NEFF executes once per load, so leftover semaphore
        # values don't matter: skip the cleanup, keep only the drain (which
        # carries the waits that guarantee all DMAs have landed).
        assert self.sems is not None
        popped = nc._tile_sem_poison_stack.pop()
        assert popped is self._sem_poison

    tile.TileContext._drain_and_barrier = _split_drain_and_barrier


def _split_multi_waits(nc):
    """This compiler's TPB instruction structs hold only one sync-wait slot
    (walrus setupSyncWait rejects more).  Tile sometimes assigns 2+ waits
    (DMA completion + slot release) to one instruction; split the excess onto
    same-engine nops inserted immediately before it."""
    import concourse.mybir as mybir

    for fn in nc.m.functions:
        for bb in fn.blocks:
            insts = list(bb.instructions)
            out = []
            changed = False
            for inst in insts:
                si = inst.sync_info
                if (
                    not isinstance(inst, mybir.InstNoOp)
                    and si is not None
                    and len(si.on_wait) > 1
                ):
                    waits = list(si.on_wait)
                    for w in waits[:-1]:
                        nop = mybir.InstNoOp(
                            name=nc.get_next_instruction_name(),
                            engine=inst.engine,
                            bass_nofuse=True,
                            sync_info=mybir.SyncInfo(on_wait=[w], on_update=[]),
                        )
                        nc.register_instruction(nop)
                        out.append(nop)
                    inst.sync_info = mybir.SyncInfo(
                        on_wait=[waits[-1]], on_update=list(si.on_update)
                    )
                    changed = True
                out.append(inst)
            if changed:
                bb.instructions = out


def _hoist_early_dmas(nc, n_act=2, n_sp=3):
    """The HWDGE rings take ~2 us to spin up after their first descriptor
    kick, and the kernel's first kicks naturally sit after the framework
    preamble (~7.2 us).  The first weight/chunk DMAs carry no semaphore
    waits, so hoist them into the 'main' preamble block right after their
    engine's register-init moves — they then kick ~1.3 us earlier and every
    early transfer (and the first real matmul) shifts forward with them."""
    import concourse.mybir as mybir

    fn = nc.m.functions[0]
    main_bb, tile_bb = fn.blocks[0], fn.blocks[1]
    for eng_name, count in (("Activation", n_act), ("SP", n_sp)):
        eng = getattr(mybir.EngineType, eng_name)
        moved = []
        for inst in list(tile_bb.instructions):
            if len(moved) >= count:
                break
            if (
                isinstance(inst, mybir.InstDMACopy)
                and inst.engine == eng
                and (inst.sync_info is None or not inst.sync_info.on_wait)
            ):
                moved.append(inst)
                tile_bb.instructions.remove(inst)
        last_mv = max(
            i
            for i, inst in enumerate(main_bb.instructions)
            if isinstance(inst, mybir.InstRegisterMove) and inst.engine == eng
        )
        main_bb.instructions[last_mv + 1 : last_mv + 1] = moved


def _make_stationaries(weight):
    """A[(r, co), idx*128 + (io*8 + jo)] = W[16a + r - io, 8b + co - jo]
    for idx = 3a + b, wherever the taps are in [0, 15)."""
    A = np.zeros((2, 3, BR, BC, BR, BC), np.float32)  # [a, b, r, co, io, jo]
    for a in range(2):
        for b in range(3):
            for io in range(BR):
                for jo in range(BC):
                    for di in range(KH):
                        r = io + di - 16 * a
                        if not 0 <= r < BR:
                            continue
                        for dj in range(KW):
                            co = jo + dj - 8 * b
                            if 0 <= co < BC:
                                A[a, b, r, co, io, jo] = weight[di, dj]
    A = A.reshape(NPASS, BR * BC, BR * BC).transpose(1, 0, 2)  # [k, idx, m]
    return np.ascontiguousarray(A.reshape(BR * BC, NPASS * BR * BC))


def _build_program(bias_val):
    import concourse.bass as bass
    import concourse.mybir as mybir
    import concourse.tile as tile

    _patch_drain()
    f32r = mybir.dt.float32r
    f32 = mybir.dt.float32
    bf16 = mybir.dt.bfloat16

    nc = bass.Bass()
    xr = nc.declare_dram_parameter(
        "xr", [128, XR_BANDS * GROUPS], bf16, isOutput=False
    )
    wa = nc.declare_dram_parameter("wa", [128, NPASS * 128], bf16, isOutput=False)
    out = nc.declare_dram_parameter("out", [NOG * 128, OG * NG], bf16, isOutput=True)

    # band index -> (chunk index, band offset within chunk)
    band_loc = []
    for k, nb in enumerate(CH_NB):
        for off in range(nb):
            band_loc.append((k, off))

    with tile.TileContext(nc) as tc:
        with (
            tc.tile_pool(name="const", bufs=1) as constp,
            tc.tile_pool(name="psum", bufs=8, space="PSUM") as psp,
            tc.tile_pool(name="op", bufs=8) as outp,
        ):
            # PE warmup fodder: memset runs right after the engine preamble,
            # so the throwaway matmuls below ramp the tensor-engine p-state
            # while the first input chunk is still in flight.
            wu = constp.tile([128, NG], bf16, tag="wu")
            nc.vector.memset(wu[:, :], 1.0)

            # Weights in bf16 so LDWEIGHTS (~100-150 ns at 1 col/cycle) hides
            # fully under the 173 ns PE pipeline window.  One combined
            # transfer on the Activation ring: per-tile descriptors issue
            # ~0.7 us apart, and the gpsimd queue is software-dynamic at
            # ~23 GB/s — both starved band 0's late passes.  The SP ring is
            # reserved for input chunks so the first band is the first
            # transfer there.
            # Split [3, 3]: finer splits serialize on per-DMA issue overhead
            # (~0.6 us each) and ring-start jitter, measured slower.
            wsplit = [(0, 3), (3, 6)]
            wtiles = []
            for wi, (lo, hi) in enumerate(wsplit):
                wt = constp.tile(
                    [128, (hi - lo) * 128], bf16, tag=f"wall{wi}", name=f"wall{wi}"
                )
                nc.scalar.dma_start(out=wt[:, :], in_=wa[:, 128 * lo : 128 * hi])
                wtiles.append(wt)
            wts = []
            for wi, (lo, hi) in enumerate(wsplit):
                for i in range(lo, hi):
                    wts.append(wtiles[wi][:, 128 * (i - lo) : 128 * (i - lo + 1)])
            chunks = []
            t0 = 0
            for k, nb in enumerate(CH_NB):
                ct = constp.tile([128, nb * GROUPS], bf16, tag=f"c{k}")
                nc.sync.dma_start(
                    out=ct[:, :],
                    in_=xr[:, GROUPS * t0 : GROUPS * (t0 + nb)],
                )
                chunks.append(ct)
                t0 += nb

            def rhs(tb, b):
                k, off = band_loc[tb]
                lo = GROUPS * off + b
                return chunks[k][:, lo : lo + NG]

            ps_warm = psp.tile([128, NG], f32, tag="ps")
            for _ in range(N_WARMUP):
                nc.tensor.matmul(
                    ps_warm[:, :], wu[:, :128], wu[:, :], start=True, stop=True
                )

            seq = [(t, idx) for t in range(NBAND) for idx in range(NPASS)]

            pss = {}
            ot = None
            for t, idx in seq:
                if t == NBAND - 1 and idx > 0:
                    continue  # emitted below by the (t, 0) handler
                if idx == 0:
                    pss[t] = psp.tile([128, NG], f32, tag="ps", name=f"ps{t}")
                if t == NBAND - 1 and idx == 0:
                    # Last band: two column halves, each evac'd and stored as
                    # soon as its half-group retires, so the kernel tail is
                    # one 64 KB store instead of a 128 KB one behind a full
                    # 512-col evac.
                    G, q = divmod(t, OG)
                    HN = NG // 2
                    for h in range(2):
                        for idx2 in range(NPASS):
                            a, b = divmod(idx2, 3)
                            nc.tensor.matmul(
                                pss[t][:, h * HN : (h + 1) * HN],
                                wts[idx2],
                                rhs(t + a, b)[:, h * HN : (h + 1) * HN],
                                start=(idx2 == 0),
                                stop=(idx2 == NPASS - 1),
                            )
                        nc.vector.tensor_scalar_add(
                            ot[:, q * NG + h * HN : q * NG + (h + 1) * HN],
                            pss[t][:, h * HN : (h + 1) * HN],
                            bias_val,
                        )
                        nc.sync.dma_start(
                            out=out[
                                128 * G : 128 * (G + 1),
                                NG * q + h * HN : NG * q + (h + 1) * HN,
                            ],
                            in_=ot[:, q * NG + h * HN : q * NG + (h + 1) * HN],
                        )
                    del pss[t]
                    continue
                a, b = divmod(idx, 3)
                nc.tensor.matmul(
                    pss[t][:, :],
                    wts[idx],
                    rhs(t + a, b),
                    start=(idx == 0),
                    stop=(idx == NPASS - 1),
                )
                if idx == NPASS - 1:
                    G, q = divmod(t, OG)
                    if q == 0:
                        ot = outp.tile([128, OG * NG], bf16, tag="ot", name=f"ot{G}")
                    nc.vector.tensor_scalar_add(
                        ot[:, q * NG : (q + 1) * NG], pss[t][:, :], bias_val
                    )
                    # Output rides both HWDGE rings: the SP ring is idle once
                    # input chunks finish (~24 us), so the second half of the
                    # bands stores there — the Activation ring then has no
                    # end-of-run backlog trailing the last matmul.
                    store_eng = nc.scalar if t < NBAND // 2 else nc.sync
                    if G == NOG - 1:
                        # store the last group per-band for a short tail
                        store_eng.dma_start(
                            out=out[128 * G : 128 * (G + 1), NG * q : NG * (q + 1)],
                            in_=ot[:, q * NG : (q + 1) * NG],
                        )
                    elif q == OG - 1:
                        store_eng.dma_start(
                            out=out[128 * G : 128 * (G + 1), :], in_=ot[:, :]
                        )
                    del pss[t]

    _split_multi_waits(nc)
    _hoist_early_dmas(nc)
    return nc


def kernel(x, weight, bias):
    global LAST_RESULT
    from concourse.bass_utils import run_bass_kernel_spmd

    import ml_dtypes

    bf16 = np.dtype(ml_dtypes.bfloat16)
    x = np.ascontiguousarray(np.asarray(x, dtype=np.float32))
    weight = np.asarray(weight, dtype=np.float32)
    bias = np.asarray(bias, dtype=np.float32)

    # Host-side zero padding: PAD on top/left, plus enough extra rows/cols
    # that every core's fixed-size slice stays in bounds.
    xpad = np.zeros((XPAD_R, XPAD_C), np.float32)
    xpad[PAD : PAD + H, PAD : PAD + W] = x
    A = _make_stationaries(weight)

    nc = _build_program(float(bias[0]))
    in_maps = []
    for c in range(NCORES):
        sl = xpad[ROWS_PC * c : ROWS_PC * c + XR_BANDS * BR]  # [528, 4128]
        xrc = (
            sl.reshape(XR_BANDS, BR, GROUPS, BC)
            .transpose(1, 3, 0, 2)  # [r, co, band, g]
            .reshape(128, XR_BANDS * GROUPS)
        )
        in_maps.append({"xr": np.ascontiguousarray(xrc.astype(bf16)), "wa": A.astype(bf16)})
    res = run_bass_kernel_spmd(
        nc,
        in_maps,
        list(range(NCORES)),
        trace=bool(os.environ.get("CONV_TRACE")),
    )
    LAST_RESULT = res

    full = np.empty((NCORES * ROWS_PC, NG * BC), np.float32)
    for c in range(NCORES):
        oc = np.asarray(res.results[c]["out"]).astype(np.float32)  # [NOG*128, OG*NG]
        full[ROWS_PC * c : ROWS_PC * (c + 1)] = (
            oc.reshape(NOG, BR, BC, OG, NG)
            .transpose(0, 3, 1, 4, 2)  # [G, q, io, n, jo]
            .reshape(ROWS_PC, NG * BC)
        )
    return np.ascontiguousarray(full[:OUT, :OUT]).astype(np.float32)


# revision 50
# speedup vs baseline: 1.0243x; 1.0243x over previous
"""Trainium2 Bass kernel: single-channel 15x15 cross-correlation (pad=1,
stride=1) of a 4096x4096 fp32 image, + scalar bias.

Strategy: 2D-patch packing, 6 matmul passes per 128-pixel block
---------------------------------------------------------------
The image is space-to-depth'd on the host: partition dim packs a 16x8
(row x col) patch, so SBUF column 516*t + g holds xpad[16t + r, 8g + co]
at partition r*8 + co.  An output block of 16x8 = 128 pixels out[(io,
jo)] at column group n accumulates 6 matmuls (a in {0,1} row-patches x
b in {0,1,2} col-shifts): stationary A_ab[(r, co), (io, jo)] =
W[16a + r - io, 8b + co - jo], moving operand = input band t+a shifted
by b groups.  Each (di, dj) tap appears in exactly one (a, b, r, co), so
6 passes replace the 15 banded passes of the naive Toeplitz scheme (the
6-pass count is optimal: a block needs (16+14)*(8+14) = 660 distinct
inputs and a stream column carries at most 128).

Work per core: 32 bands (512 output rows) x 512 column groups -> 192
matmuls of [K=128, M=128] x [128, 512], all operands bf16 (PSUM f32,
rel err ~2.6e-3).  bf16 matters twice: LDWEIGHTS drops to ~100 ns and
hides fully under the ~173 ns PE pipeline window (fp32r weight loads
are 2 B/col and stick out, 234->280 ns per matmul), and wire traffic
halves.  Steady state measured 215 ns per matmul = 512 cols at 2.4 GHz.

Scheduling details, all trace-driven:
  - input rides the SP HWDGE ring as 7 size-ramped chunk DMAs (the
    33 per-band DMAs' ring-slot semaphores cross-paced with the output
    ring and stalled the PE; big chunks also move at ~300 GB/s vs
    ~90 GB/s for 1 KB-element transfers);
  - weights ride the Activation ring as two 98 KB transfers sized so
    wts[0..2] land just before the first real matmul needs them;
  - all 8 PSUM banks cycle as single-band accumulators (bufs=8), so
    the PE never waits on the DVE bias-add evacuation;
  - output stores go 4 bands per DMA (4 KB elements) split across both
    HWDGE rings, and the last band stores as two 64 KB halves to keep
    the post-compute tail short;
  - throwaway matmuls on memset data bridge the fixed ~7 us engine
    preamble until input lands, so the PE p-state (0.65/1.2/2.4 GHz
    ramp) is at full clock when the first real matmul issues.
Host pre-pads/reshapes inputs and un-shuffles outputs; halos ride in
each core's input slice, no collectives.  Measured: 146.8 us (banded
baseline) -> ~58 us, rel err 2.6e-3.
"""

import os

import numpy as np

KH = KW = 15
PAD = 1
H = W = 4096
OUT = H + 2 * PAD - KH + 1  # 4084
NCORES = 8
BR = 16  # output rows per band (and patch rows)
BC = 8  # col-group width (and patch cols)
NBAND = 32  # bands per core -> 512 output rows per core
NG = 512  # output col groups per stream (512*8 = 4096 >= 4084 cols)
GROUPS = NG + 4  # col groups per band (2 extra for b-shift, pad to 516)
NPASS = 6  # 2 row-patches x 3 col-shifts
ROWS_PC = NBAND * BR  # 512 output rows per core
XR_BANDS = NBAND + 1  # input bands per core (one extra for the halo)
XPAD_R = NCORES * ROWS_PC + BR  # 4112 padded input rows
XPAD_C = GROUPS * BC  # 4128 padded input cols
CH_NB = [2, 2, 3, 4, 8, 8, 6]  # input-chunk sizes in bands (sum = 33)
N_WARMUP = 6
OG = 4  # bands per output-store group (4 KB DMA elements for ring rate)
NOG = NBAND // OG

LAST_RESULT = None  # BassKernelResults of the most recent run (for test.py)


def _patch_drain():
    """walrus's CTRL_NO instruction struct holds very few semaphore waits;
    Tile's kernel-tail drain aggregates one wait per logical processor and
    overflows it.  Spread the waits across 1-wait-per-nop SP instructions."""
    import concourse.mybir as mybir
    import concourse.tile as tile
    from concourse.vector_clock import ScopedClock

    def _split_drain_and_barrier(self, tick_clock, wait_clock):
        nc = self.nc
        probe = nc.sync.nop(nofuse=True)
        wait_clock.add_sem_waits(
            probe.ins, ScopedClock({None: tick_clock.global_clock})
        )
        si = probe.ins.sync_info
        if si is not None and len(si.on_wait) > 1:
            waits = list(si.on_wait)
            probe.ins.sync_info = mybir.SyncInfo(
                on_wait=waits[:1], on_update=list(si.on_update)
            )
            for w in waits[1:]:
                extra = nc.sync.nop(nofuse=True)
                extra.ins.sync_info = mybir.SyncInfo(on_wait=[w], on_update=[])
        nc.sync.drain()
        # The stock exit path does barrier -> semaphore cleanup -> barrier
        # (~8us).  This NEFF executes once per load, so leftover semaphore
        # values don't matter: skip the cleanup, keep only the drain (which
        # carries the waits that guarantee all DMAs have landed).
        assert self.sems is not None
        popped = nc._tile_sem_poison_stack.pop()
        assert popped is self._sem_poison

    tile.TileContext._drain_and_barrier = _split_drain_and_barrier


def _split_multi_waits(nc):
    """This compiler's TPB instruction structs hold only one sync-wait slot
    (walrus setupSyncWait rejects more).  Tile sometimes assigns 2+ waits
    (DMA completion + slot release) to one instruction; split the excess onto
    same-engine nops inserted immediately before it."""
    import concourse.mybir as mybir

    for fn in nc.m.functions:
        for bb in fn.blocks:
            insts = list(bb.instructions)
            out = []
            changed = False
            for inst in insts:
                si = inst.sync_info
                if (
                    not isinstance(inst, mybir.InstNoOp)
                    and si is not None
                    and len(si.on_wait) > 1
                ):
                    waits = list(si.on_wait)
                    for w in waits[:-1]:
                        nop = mybir.InstNoOp(
                            name=nc.get_next_instruction_name(),
                            engine=inst.engine,
                            bass_nofuse=True,
                            sync_info=mybir.SyncInfo(on_wait=[w], on_update=[]),
                        )
                        nc.register_instruction(nop)
                        out.append(nop)
                    inst.sync_info = mybir.SyncInfo(
                        on_wait=[waits[-1]], on_update=list(si.on_update)
                    )
                    changed = True
                out.append(inst)
            if changed:
                bb.instructions = out


def _hoist_early_dmas(nc, n_act=2, n_sp=3):
    """The HWDGE rings take ~2 us to spin up after their first descriptor
    kick, and the kernel's first kicks naturally sit after the framework
    preamble (~7.2 us).  The first weight/chunk DMAs carry no semaphore
    waits, so hoist them into the 'main' preamble block right after their
    engine's register-init moves — they then kick ~1.3 us earlier and every
    early transfer (and the first real matmul) shifts forward with them."""
    import concourse.mybir as mybir

    fn = nc.m.functions[0]
    main_bb, tile_bb = fn.blocks[0], fn.blocks[1]
    for eng_name, count in (("Activation", n_act), ("SP", n_sp)):
        eng = getattr(mybir.EngineType, eng_name)
        moved = []
        for inst in list(tile_bb.instructions):
            if len(moved) >= count:
                break
            if (
                isinstance(inst, mybir.InstDMACopy)
                and inst.engine == eng
                and (inst.sync_info is None or not inst.sync_info.on_wait)
            ):
                moved.append(inst)
                tile_bb.instructions.remove(inst)
        # Insert between the engine's preamble InstDrain and its barrier
        # EventSemaphore: the drain has already seen empty queues, so the
        # barrier rendezvous proceeds while our transfers are in flight.
        # (Inserting before the drain stalls the whole preamble barrier
        # until the transfers complete — measured +2 us.)
        last_drain = max(
            i
            for i, inst in enumerate(main_bb.instructions)
            if isinstance(inst, mybir.InstDrain) and inst.engine == eng
        )
        main_bb.instructions[last_drain + 1 : last_drain + 1] = moved


def _make_stationaries(weight):
    """A[(r, co), idx*128 + (io*8 + jo)] = W[16a + r - io, 8b + co - jo]
    for idx = 3a + b, wherever the taps are in [0, 15)."""
    A = np.zeros((2, 3, BR, BC, BR, BC), np.float32)  # [a, b, r, co, io, jo]
    for a in range(2):
        for b in range(3):
            for io in range(BR):
                for jo in range(BC):
                    for di in range(KH):
                        r = io + di - 16 * a
                        if not 0 <= r < BR:
                            continue
                        for dj in range(KW):
                            co = jo + dj - 8 * b
                            if 0 <= co < BC:
                                A[a, b, r, co, io, jo] = weight[di, dj]
    A = A.reshape(NPASS, BR * BC, BR * BC).transpose(1, 0, 2)  # [k, idx, m]
    return np.ascontiguousarray(A.reshape(BR * BC, NPASS * BR * BC))


def _build_program(bias_val):
    import concourse.bass as bass
    import concourse.mybir as mybir
    import concourse.tile as tile

    _patch_drain()
    f32r = mybir.dt.float32r
    f32 = mybir.dt.float32
    bf16 = mybir.dt.bfloat16

    nc = bass.Bass()
    xr = nc.declare_dram_parameter(
        "xr", [128, XR_BANDS * GROUPS], bf16, isOutput=False
    )
    wa = nc.declare_dram_parameter("wa", [128, NPASS * 128], bf16, isOutput=False)
    out = nc.declare_dram_parameter("out", [NOG * 128, OG * NG], bf16, isOutput=True)

    # band index -> (chunk index, band offset within chunk)
    band_loc = []
    for k, nb in enumerate(CH_NB):
        for off in range(nb):
            band_loc.append((k, off))

    with tile.TileContext(nc) as tc:
        with (
            tc.tile_pool(name="const", bufs=1) as constp,
            tc.tile_pool(name="psum", bufs=8, space="PSUM") as psp,
            tc.tile_pool(name="op", bufs=8) as outp,
        ):
            # PE warmup fodder: memset runs right after the engine preamble,
            # so the throwaway matmuls below ramp the tensor-engine p-state
            # while the first input chunk is still in flight.
            wu = constp.tile([128, NG], bf16, tag="wu")
            nc.vector.memset(wu[:, :], 1.0)

            # Weights in bf16 so LDWEIGHTS (~100-150 ns at 1 col/cycle) hides
            # fully under the 173 ns PE pipeline window.  One combined
            # transfer on the Activation ring: per-tile descriptors issue
            # ~0.7 us apart, and the gpsimd queue is software-dynamic at
            # ~23 GB/s — both starved band 0's late passes.  The SP ring is
            # reserved for input chunks so the first band is the first
            # transfer there.
            # Split [3, 3]: finer splits serialize on per-DMA issue overhead
            # (~0.6 us each) and ring-start jitter, measured slower.
            wsplit = [(0, 3), (3, 6)]
            wtiles = []
            for wi, (lo, hi) in enumerate(wsplit):
                wt = constp.tile(
                    [128, (hi - lo) * 128], bf16, tag=f"wall{wi}", name=f"wall{wi}"
                )
                nc.scalar.dma_start(out=wt[:, :], in_=wa[:, 128 * lo : 128 * hi])
                wtiles.append(wt)
            wts = []
            for wi, (lo, hi) in enumerate(wsplit):
                for i in range(lo, hi):
                    wts.append(wtiles[wi][:, 128 * (i - lo) : 128 * (i - lo + 1)])
            chunks = []
            t0 = 0
            for k, nb in enumerate(CH_NB):
                ct = constp.tile([128, nb * GROUPS], bf16, tag=f"c{k}")
                nc.sync.dma_start(
                    out=ct[:, :],
                    in_=xr[:, GROUPS * t0 : GROUPS * (t0 + nb)],
                )
                chunks.append(ct)
                t0 += nb

            def rhs(tb, b):
                k, off = band_loc[tb]
                lo = GROUPS * off + b
                return chunks[k][:, lo : lo + NG]

            ps_warm = psp.tile([128, NG], f32, tag="ps")
            for _ in range(N_WARMUP):
                nc.tensor.matmul(
                    ps_warm[:, :], wu[:, :128], wu[:, :], start=True, stop=True
                )

            seq = [(t, idx) for t in range(NBAND) for idx in range(NPASS)]

            pss = {}
            ot = None
            for t, idx in seq:
                if t == NBAND - 1 and idx > 0:
                    continue  # emitted below by the (t, 0) handler
                if idx == 0:
                    pss[t] = psp.tile([128, NG], f32, tag="ps", name=f"ps{t}")
                if t == NBAND - 1 and idx == 0:
                    # Last band: two column halves, each evac'd and stored as
                    # soon as its half-group retires, so the kernel tail is
                    # one 64 KB store instead of a 128 KB one behind a full
                    # 512-col evac.
                    G, q = divmod(t, OG)
                    HN = NG // 2
                    for h in range(2):
                        for idx2 in range(NPASS):
                            a, b = divmod(idx2, 3)
                            nc.tensor.matmul(
                                pss[t][:, h * HN : (h + 1) * HN],
                                wts[idx2],
                                rhs(t + a, b)[:, h * HN : (h + 1) * HN],
                                start=(idx2 == 0),
                                stop=(idx2 == NPASS - 1),
                            )
                        nc.vector.tensor_scalar_add(
                            ot[:, q * NG + h * HN : q * NG + (h + 1) * HN],
                            pss[t][:, h * HN : (h + 1) * HN],
                            bias_val,
                        )
                        nc.sync.dma_start(
                            out=out[
                                128 * G : 128 * (G + 1),
                                NG * q + h * HN : NG * q + (h + 1) * HN,
                            ],
                            in_=ot[:, q * NG + h * HN : q * NG + (h + 1) * HN],
                        )
                    del pss[t]
                    continue
                a, b = divmod(idx, 3)
                nc.tensor.matmul(
                    pss[t][:, :],
                    wts[idx],
                    rhs(t + a, b),
                    start=(idx == 0),
                    stop=(idx == NPASS - 1),
                )
                if idx == NPASS - 1:
                    G, q = divmod(t, OG)
                    if q == 0:
                        ot = outp.tile([128, OG * NG], bf16, tag="ot", name=f"ot{G}")
                    nc.vector.tensor_scalar_add(
                        ot[:, q * NG : (q + 1) * NG], pss[t][:, :], bias_val
                    )
                    # Output rides both HWDGE rings: the SP ring is idle once
                    # input chunks finish (~24 us), so the second half of the
                    # bands stores there — the Activation ring then has no
                    # end-of-run backlog trailing the last matmul.
                    store_eng = nc.scalar if t < NBAND // 2 else nc.sync
                    if G == NOG - 1:
                        # store the last group per-band for a short tail
                        store_eng.dma_start(
                            out=out[128 * G : 128 * (G + 1), NG * q : NG * (q + 1)],
                            in_=ot[:, q * NG : (q + 1) * NG],
                        )
                    elif q == OG - 1:
                        store_eng.dma_start(
                            out=out[128 * G : 128 * (G + 1), :], in_=ot[:, :]
                        )
                    del pss[t]

    _split_multi_waits(nc)
    _hoist_early_dmas(nc)
    return nc


def kernel(x, weight, bias):
    global LAST_RESULT
    from concourse.bass_utils import run_bass_kernel_spmd

    import ml_dtypes

    bf16 = np.dtype(ml_dtypes.bfloat16)
    x = np.ascontiguousarray(np.asarray(x, dtype=np.float32))
    weight = np.asarray(weight, dtype=np.float32)
    bias = np.asarray(bias, dtype=np.float32)

    # Host-side zero padding: PAD on top/left, plus enough extra rows/cols
    # that every core's fixed-size slice stays in bounds.
    xpad = np.zeros((XPAD_R, XPAD_C), np.float32)
    xpad[PAD : PAD + H, PAD : PAD + W] = x
    A = _make_stationaries(weight)

    nc = _build_program(float(bias[0]))
    in_maps = []
    for c in range(NCORES):
        sl = xpad[ROWS_PC * c : ROWS_PC * c + XR_BANDS * BR]  # [528, 4128]
        xrc = (
            sl.reshape(XR_BANDS, BR, GROUPS, BC)
            .transpose(1, 3, 0, 2)  # [r, co, band, g]
            .reshape(128, XR_BANDS * GROUPS)
        )
        in_maps.append({"xr": np.ascontiguousarray(xrc.astype(bf16)), "wa": A.astype(bf16)})
    res = run_bass_kernel_spmd(
        nc,
        in_maps,
        list(range(NCORES)),
        trace=bool(os.environ.get("CONV_TRACE")),
    )
    LAST_RESULT = res

    full = np.empty((NCORES * ROWS_PC, NG * BC), np.float32)
    for c in range(NCORES):
        oc = np.asarray(res.results[c]["out"]).astype(np.float32)  # [NOG*128, OG*NG]
        full[ROWS_PC * c : ROWS_PC * (c + 1)] = (
            oc.reshape(NOG, BR, BC, OG, NG)
            .transpose(0, 3, 1, 4, 2)  # [G, q, io, n, jo]
            .reshape(ROWS_PC, NG * BC)
        )
    return np.ascontiguousarray(full[:OUT, :OUT]).astype(np.float32)
